# revision 1
# baseline (speedup 1.0000x reference)
"""Chamfer distance loss kernel for Trainium2 (8 NeuronCores).

Problem: template/source (4, 8192, 3) f32. For each batch b:
  d[n,m] = |t_n|^2 - 2 t_n.s_m + |s_m|^2
  loss_b = mean_n min_m d + mean_m min_n d ; output = mean_b loss_b (scalar).

Sharding: core c handles (batch = c//2, template-row-half = c%2):
4096 template rows x all 8192 source points per core; the 8 per-core
partials are combined on the host (the cross-core reduction is a handful
of tiny vectors).

Per core, each [128 rows x 512 cols] distance tile is produced directly
in PSUM by ONE augmented matmul: d = |t|^2 - 2 t.s + |s|^2 becomes a
K=24 contraction of bf16 3-way value splits (a = a1+a2+a3 exactly in
bf16 parts; products kept to O(2^-27)), giving fp32-grade distances at
1 cycle/row — 4x faster than native fp32 matmuls. ScalarE evacuates
PSUM to SBUF as fp16 (quantizing d to fp16 before the min is safe: the
final rel err stays at ~4e-5), VectorE runs all min-accumulation in
fp16 at its 2x packed mode: a per-row-tile running min over column
groups (folded 2048->512 with 2x tensor_tensor ops before the 1x-only
tensor_reduce), and a per-column running min over row tiles kept in two
half-range accumulators so the first half's output DMA overlaps the
second half's compute. The 128-partition fold of the column minima and
all tiny means/sums happen on the host.
"""
import os
import sys

sys.path.insert(0, "/opt/trn_rl_repo")

from contextlib import ExitStack

import numpy as np

import concourse.bass as bass
import concourse.tile as tile
from concourse import mybir
from concourse.bass_utils import run_bass_kernel_spmd

# ---------------------------------------------------------------------------
# The walrus build in this container rejects instructions carrying more than
# one sync-wait command. After Tile scheduling, split any multi-wait
# instruction: keep the first wait on it and hoist the rest onto standalone
# EventSemaphore instructions inserted just before it (same engine, so
# per-engine program order makes the waits execute first).
import bass_rust as _br


def split_multi_waits(nc):
    n_new = 0
    for fn in nc.m.functions:
        for blk in fn.blocks:
            insts = list(blk.instructions)
            out = []
            changed = False
            for inst in insts:
                si = inst.sync_info
                waits = list(si.on_wait) if si is not None and si.on_wait else []
                if len(waits) > 1:
                    for w in waits[:-1]:
                        ev = _br.InstEventSemaphore(
                            name=f"I-waitsplit-{n_new}", ins=[], outs=[]
                        )
                        n_new += 1
                        ev.engine = inst.engine
                        ev.sync_info = _br.SyncInfo(on_wait=[w], on_update=[])
                        out.append(ev)
                    si.on_wait = [waits[-1]]
                    changed = True
                out.append(inst)
            if changed:
                blk.instructions = out
# ---------------------------------------------------------------------------

import ml_dtypes

F32 = mybir.dt.float32
F32R = mybir.dt.float32r
F16 = mybir.dt.float16
BF16 = mybir.dt.bfloat16
MIN = mybir.AluOpType.min
BF16NP = ml_dtypes.bfloat16

B, N, M, D = 4, 8192, 8192, 3
R = N // 2      # template rows per core
NCORES = 8
GROUP = 2048    # psum group: 4 matmuls of 512

# "dekker": bf16 3-way-split matmuls, K=24 (1 cycle/row; error at the fp16
#           cast floor ~3.7e-5 final rel err — same as exact-fp32 matmuls)
# "f32"   : exact fp32 matmuls, K=5 (4 cycles/row, slowest, exact)
# "f32r"  : float32r matmuls, K=5 (fast but ~tf32 precision: too coarse)
MM_MODE = os.environ.get("CHAMFER_MM_MODE", "dekker")
K_BY_MODE = {"dekker": 24, "f32": 5, "f32r": 5}
K = K_BY_MODE[MM_MODE]

# offload every Nth eligible colacc min to the DMA compute-copy engine
# (gpsimd SWDGE accum: out = min(in, out)); 0 disables
DMAMIN_MOD = int(os.environ.get("CHAMFER_DMAMIN_MOD", "0"))



def build_program(rows=R, cols=M, mm_mode=MM_MODE, split_waits=True):
    row_tiles = rows // 128
    ngroups = cols // GROUP
    k = K_BY_MODE[mm_mode]
    nc = bass.Bass("TRN2", target_bir_lowering=False, debug=False)
    mm_dt = {"dekker": BF16, "f32": F32, "f32r": F32R}[mm_mode]
    lhsT = nc.dram_tensor("lhsT_aug", [k, rows], mm_dt, kind="ExternalInput").ap()
    rhs = nc.dram_tensor("rhs_aug", [k, cols], mm_dt, kind="ExternalInput").ap()
    o_rm = nc.dram_tensor(
        "out_rowmin", [128, row_tiles], F32, kind="ExternalOutput"
    ).ap()
    o_cm = nc.dram_tensor(
        "out_colmin", [2, 128, cols], F16, kind="ExternalOutput"
    ).ap()

    def vmin(out_ap, a_ap, b_ap):
        nc.vector.tensor_tensor(out_ap, a_ap, b_ap, op=MIN)

    with tile.TileContext(nc) as tc, ExitStack() as ctx:
        consts = ctx.enter_context(tc.tile_pool(name="consts", bufs=1))
        psum_pool = ctx.enter_context(tc.tile_pool(name="psum", bufs=2, space="PSUM"))
        cast_pool = ctx.enter_context(tc.tile_pool(name="cast", bufs=6))
        rowacc_pool = ctx.enter_context(tc.tile_pool(name="rowacc", bufs=4))
        rfold_pool = ctx.enter_context(tc.tile_pool(name="rfold", bufs=2))
        accs = ctx.enter_context(tc.tile_pool(name="accs", bufs=1))

        # warm the ACT function-table (its ~2.7us load overlaps input DMA)
        warm = consts.tile([1, 16], F16)
        nc.vector.memset(warm[:], 0.0)
        nc.scalar.copy(warm[:], warm[:])

        lhsT_sb = consts.tile([k, rows], mm_dt)
        nc.sync.dma_start(lhsT_sb[:], lhsT)
        rhs_sb = consts.tile([k, cols], mm_dt)
        # chunked so the first matmuls only wait on their own column range
        for q in range(ngroups):
            nc.sync.dma_start(
                rhs_sb[:, q * GROUP:(q + 1) * GROUP],
                rhs[:, q * GROUP:(q + 1) * GROUP],
            )

        # two colacc halves: the first half's accumulator is final midway
        # through the loop, so its output DMA overlaps the second half
        colaccA = accs.tile([128, cols], F16)
        colaccB = accs.tile([128, cols], F16)
        rowminb = accs.tile([128, row_tiles], F32)
        half_tiles = max(row_tiles // 2, 1)

        for i in range(row_tiles):
            lh = lhsT_sb[:, i * 128:(i + 1) * 128]
            cacc = colaccA if i < half_tiles else colaccB
            first = i == 0 or i == half_tiles
            rowacc = rowacc_pool.tile([128, GROUP], F16)
            for g in range(ngroups):
                ps = psum_pool.tile([128, GROUP], F32)
                for jj in range(4):
                    c0 = g * GROUP + jj * 512
                    nc.tensor.matmul(
                        ps[:, jj * 512:(jj + 1) * 512], lh,
                        rhs_sb[:, c0:c0 + 512],
                        start=True, stop=True,
                    )
                ca = cacc[:, g * GROUP:(g + 1) * GROUP]
                if first:
                    # first row tile of a half initializes colacc from ACT
                    if i == 0 and g == 0:
                        # lead-in: cast in 1024-col halves; bank-level psum
                        # deps let the first cast start after 2 of 4 matmuls
                        for h in (0, 1):
                            sub = slice(h * 1024, (h + 1) * 1024)
                            nc.scalar.copy(ca[:, sub], ps[:, sub])
                            nc.vector.tensor_copy(rowacc[:, sub], ca[:, sub])
                        continue
                    nc.scalar.copy(ca, ps[:])
                    if g == 0:
                        nc.vector.tensor_copy(rowacc[:], ca)
                    else:
                        vmin(rowacc[:], ca, rowacc[:])
                elif g == 0:
                    # ACT writes rowacc directly; colacc mins against it
                    nc.scalar.copy(rowacc[:], ps[:])
                    vmin(ca, rowacc[:], ca)
                else:
                    cst = cast_pool.tile([128, GROUP], F16)
                    nc.scalar.copy(cst[:], ps[:])
                    vmin(rowacc[:], cst[:], rowacc[:])
                    if DMAMIN_MOD and (i * ngroups + g) % DMAMIN_MOD == 0:
                        nc.gpsimd.dma_start(ca, cst[:], accum_op=MIN)
                    else:
                        vmin(ca, cst[:], ca)
                if i == row_tiles - 1 and row_tiles > 1:
                    # this column group of colaccB is final: ship it now so
                    # only the last chunk's DMA is exposed at the tail
                    nc.sync.dma_start(
                        o_cm[1][:, g * GROUP:(g + 1) * GROUP], ca
                    )
            # pre-fold with 2x tt ops before the 1x-only reduce; fold into a
            # separate tile so rowacc's buffer is released after one op
            rfold = rfold_pool.tile([128, 1024], F16)
            vmin(rfold[:], rowacc[:, 0:1024], rowacc[:, 1024:2048])
            vmin(rfold[:, 0:512], rfold[:, 0:512], rfold[:, 512:1024])
            nc.vector.tensor_reduce(
                rowminb[:, i:i + 1], rfold[:, 0:512],
                axis=mybir.AxisListType.X, op=MIN,
            )
            if i == half_tiles - 1 and row_tiles > 1:
                # colaccA final: ship it while the second half computes
                nc.sync.dma_start(o_cm[0], colaccA[:])

        if row_tiles == 1:
            nc.sync.dma_start(o_cm[0], colaccA[:])
            nc.sync.dma_start(o_cm[1], colaccA[:])
        nc.sync.dma_start(o_rm, rowminb[:])
    if split_waits:
        split_multi_waits(nc)  # CoreSim can't model the injected waits
    return nc


_program_cache = {}


def _get_program():
    key = (R, M, MM_MODE)
    if key not in _program_cache:
        _program_cache[key] = build_program()
    return _program_cache[key]


def enable_profiling():
    """Wire up the NTFF profiling hook (the image's antenv lacks
    antenv.axon_hooks) and neuter the credential-requiring artifact upload.
    Needed only when tracing (BASS_TRACE=1); harmless otherwise."""
    import types
    import antenv
    import concourse.bass_utils as _bu

    if "antenv.axon_hooks" not in sys.modules:
        hooks = types.ModuleType("antenv.axon_hooks")
        hooks._h = None
        hooks.set_axon_ntff_profile_hook = lambda h: setattr(hooks, "_h", h)
        hooks.get_axon_ntff_profile_hook = lambda: hooks._h
        sys.modules["antenv.axon_hooks"] = hooks
        antenv.axon_hooks = hooks
        try:
            from trn_agent_boot.trn_boot import _ntff_profile_via_ctypes

            hooks.set_axon_ntff_profile_hook(
                _ntff_profile_via_ctypes("/opt/axon/libaxon_pjrt.so")
            )
        except Exception:
            pass
    _bu.upload_artifacts = lambda tmpdir: f"local:{tmpdir}"


if os.environ.get("BASS_TRACE"):
    try:
        enable_profiling()
    except Exception:
        pass


def _aug_f32(t, s):
    """K=5 fp32 augmentation: d = |t|^2 - 2 t.s + |s|^2 in one matmul."""
    rows, cols = t.shape[0], s.shape[0]
    lhsT = np.empty((5, rows), np.float32)
    lhsT[0:3] = t.T
    lhsT[3] = (t * t).sum(axis=1)
    lhsT[4] = 1.0
    rhs = np.empty((5, cols), np.float32)
    rhs[0:3] = -2.0 * s.T
    rhs[3] = 1.0
    rhs[4] = (s * s).sum(axis=1)
    return lhsT, rhs


def _split3(x):
    x1 = x.astype(BF16NP)
    r = x - x1.astype(np.float32)
    x2 = r.astype(BF16NP)
    x3 = (r - x2.astype(np.float32)).astype(BF16NP)
    return x1, x2, x3


def _aug_dekker(t, s):
    """K=24 bf16 3-way-split augmentation. Each fp32 value a = a1+a2+a3 in
    bf16 parts; products kept to O(2^-27): a1b1, a1b2, a2b1, a1b3, a3b1,
    a2b2. PE cost is free-dim cycles only, so K=24 runs as fast as K=5."""
    rows, cols = t.shape[0], s.shape[0]
    t1, t2, t3 = _split3(t)
    s1, s2, s3 = _split3(-2.0 * s)
    n1, n2, n3 = _split3((t * t).sum(axis=1))
    m1, m2, m3 = _split3((s * s).sum(axis=1))
    one = np.ones((), BF16NP)
    lhsT = np.empty((24, rows), BF16NP)
    for j, part in enumerate((t1, t1, t2, t1, t3, t2)):
        lhsT[3 * j:3 * j + 3] = part.T
    lhsT[18] = n1
    lhsT[19] = n2
    lhsT[20] = n3
    lhsT[21:24] = one
    rhs = np.empty((24, cols), BF16NP)
    for j, part in enumerate((s1, s2, s1, s3, s1, s2)):
        rhs[3 * j:3 * j + 3] = part.T
    rhs[18:21] = one
    rhs[21] = m1
    rhs[22] = m2
    rhs[23] = m3
    return lhsT, rhs


def make_in_maps(template, source, mm_mode=MM_MODE):
    template = np.asarray(template, dtype=np.float32)
    source = np.asarray(source, dtype=np.float32)
    aug = _aug_dekker if mm_mode == "dekker" else _aug_f32
    in_maps = []
    for c in range(NCORES):
        b, h = c // 2, c % 2
        t = template[b, h * R:(h + 1) * R]      # [R, 3]
        s = source[b]                            # [M, 3]
        lhsT, rhs = aug(t, s)
        in_maps.append(
            {"lhsT_aug": np.ascontiguousarray(lhsT),
             "rhs_aug": np.ascontiguousarray(rhs)}
        )
    return in_maps


last_results = None  # BassKernelResults of the most recent kernel() call


def kernel(template, source):
    global last_results
    nc = _get_program()
    in_maps = make_in_maps(template, source)
    res = run_bass_kernel_spmd(nc, in_maps, list(range(NCORES)))
    last_results = res

    per_batch = np.zeros(B, dtype=np.float64)
    for b in range(B):
        r0 = res.results[2 * b + 0]
        r1 = res.results[2 * b + 1]
        rowsum = (
            r0["out_rowmin"].astype(np.float64).sum()
            + r1["out_rowmin"].astype(np.float64).sum()
        )
        cost_p0_p1 = rowsum / N
        cm = np.minimum(
            r0["out_colmin"].astype(np.float32).reshape(-1, M).min(axis=0),
            r1["out_colmin"].astype(np.float32).reshape(-1, M).min(axis=0),
        )
        cost_p1_p0 = cm.astype(np.float64).mean()
        per_batch[b] = cost_p0_p1 + cost_p1_p0
    return np.float32(per_batch.mean())



# revision 5
# speedup vs baseline: 2.7329x; 2.7329x over previous
"""Chamfer distance loss kernel for Trainium2 (8 NeuronCores).

Problem: template/source (4, 8192, 3) f32. For each batch b:
  d[n,m] = |t_n|^2 - 2 t_n.s_m + |s_m|^2
  loss_b = mean_n min_m d + mean_m min_n d ; output = mean_b loss_b (scalar).

Strategy: z-sorted banding. Both point sets are sorted by z on the host
(per batch). A 128-row template tile then only needs a contiguous band
of ~2048 z-sorted source columns: nearest neighbors are z-close for all
but a handful of radial outliers. This cuts the distance-matrix volume
4x (32 tiles x 2048 cols per core instead of 32 x 8192).

The banding is made EXACT by a host-side patch: a banded row-min can
only be wrong if the true nearest neighbor lies outside the band, which
implies d >= (z-gap to the band edge)^2. Rows/cols whose banded min
exceeds that bound are recomputed exactly in numpy (a few hundred per
batch, milliseconds). Means are permutation-invariant so the sort needs
no undoing.

Sharding: core c = (batch c//2, sorted-template-half c%2); each core's
source input is a pre-shifted 6144-rank window of the sorted source
(rank range [4096h-960, 4096h+5184)) padded with far-away sentinel
points, which keeps one SPMD program valid for both halves: tile i
always scans virtual columns [128i, 128i+2048).

Per core, each [128 x 512] distance tile is produced in PSUM by ONE
augmented matmul: d = |t|^2 - 2 t.s + |s|^2 as a K=24 contraction of
bf16 3-way value splits (fp32-grade distances at 1 cycle/row). ScalarE
casts each 2048-col PSUM group to fp16 once; VectorE does the row-min
reduce and the column-min accumulate in fp16 2x packed mode. Column
minima ship to HBM in 1024-col chunks as soon as their last touching
tile has passed, overlapping output DMA with compute.
"""
import os
import sys

sys.path.insert(0, "/opt/trn_rl_repo")

from contextlib import ExitStack

import numpy as np

import concourse.bass as bass
import concourse.tile as tile
from concourse import mybir
from concourse.bass_utils import run_bass_kernel_spmd

# ---------------------------------------------------------------------------
# The walrus build in this container rejects instructions carrying more than
# one sync-wait command. After Tile scheduling, split any multi-wait
# instruction: keep the first wait on it and hoist the rest onto standalone
# EventSemaphore instructions inserted just before it (same engine, so
# per-engine program order makes the waits execute first).
import bass_rust as _br


def split_multi_waits(nc):
    n_new = 0
    for fn in nc.m.functions:
        for blk in fn.blocks:
            insts = list(blk.instructions)
            out = []
            changed = False
            for inst in insts:
                si = inst.sync_info
                waits = list(si.on_wait) if si is not None and si.on_wait else []
                if len(waits) > 1:
                    for w in waits[:-1]:
                        ev = _br.InstEventSemaphore(
                            name=f"I-waitsplit-{n_new}", ins=[], outs=[]
                        )
                        n_new += 1
                        ev.engine = inst.engine
                        ev.sync_info = _br.SyncInfo(on_wait=[w], on_update=[])
                        out.append(ev)
                    si.on_wait = [waits[-1]]
                    changed = True
                out.append(inst)
            if changed:
                blk.instructions = out
# ---------------------------------------------------------------------------

import ml_dtypes

F32 = mybir.dt.float32
F16 = mybir.dt.float16
BF16 = mybir.dt.bfloat16
MIN = mybir.AluOpType.min
BF16NP = ml_dtypes.bfloat16

B, N, M, D = 4, 8192, 8192, 3
HALF = N // 2       # template rows per core
NCORES = 8
K = 24              # dekker-split contraction depth
W = 960             # band half-width in source ranks
BAND = 2048         # columns per tile band (= 128 + 2*W)
VCOLS = HALF + 2 * W + 128  # virtual source columns per core = 6144
ROW_TILES = HALF // 128     # 32
BIG = 60000.0       # fp16-safe "+inf" for min accumulators
SENT_SQ = 30000.0   # sentinel |s|^2: d at padded columns never wins a min


def build_program(split_waits=True):
    nc = bass.Bass("TRN2", target_bir_lowering=False, debug=False)
    lhsT = nc.dram_tensor("lhsT_aug", [K, HALF], BF16, kind="ExternalInput").ap()
    rhs = nc.dram_tensor("rhs_aug", [K, VCOLS], BF16, kind="ExternalInput").ap()
    o_rm = nc.dram_tensor(
        "out_rowmin", [128, ROW_TILES], F16, kind="ExternalOutput"
    ).ap()
    o_cm = nc.dram_tensor("out_colmin", [128, VCOLS], F16, kind="ExternalOutput").ap()

    def vmin(out_ap, a_ap, b_ap):
        nc.vector.tensor_tensor(out_ap, a_ap, b_ap, op=MIN)

    # last tile index that touches each 1024-col colacc chunk: ship chunks
    # as they become final so output DMA overlaps compute
    nchunks = VCOLS // 1024
    last_touch = [
        max(i for i in range(ROW_TILES)
            if 128 * i < 1024 * (k + 1) and 128 * i + BAND > 1024 * k)
        for k in range(nchunks)
    ]

    with tile.TileContext(nc) as tc, ExitStack() as ctx:
        consts = ctx.enter_context(tc.tile_pool(name="consts", bufs=1))
        psum_pool = ctx.enter_context(tc.tile_pool(name="psum", bufs=2, space="PSUM"))
        cast_pool = ctx.enter_context(tc.tile_pool(name="cast", bufs=4))
        rfold_pool = ctx.enter_context(tc.tile_pool(name="rfold", bufs=2))
        accs = ctx.enter_context(tc.tile_pool(name="accs", bufs=1))

        # warm the ACT function-table (its ~2.7us load overlaps input DMA)
        warm = consts.tile([1, 16], F16)
        nc.vector.memset(warm[:], 0.0)
        nc.scalar.copy(warm[:], warm[:])

        lhsT_sb = consts.tile([K, HALF], BF16)
        nc.sync.dma_start(lhsT_sb[:], lhsT)
        rhs_sb = consts.tile([K, VCOLS], BF16)
        # chunked so the first matmuls only wait on their own column range
        for q in range(VCOLS // 2048):
            nc.sync.dma_start(
                rhs_sb[:, q * 2048:(q + 1) * 2048],
                rhs[:, q * 2048:(q + 1) * 2048],
            )

        colacc = accs.tile([128, VCOLS], F16)
        nc.vector.memset(colacc[:], BIG)
        rowminb = accs.tile([128, ROW_TILES], F16)

        for i in range(ROW_TILES):
            lh = lhsT_sb[:, i * 128:(i + 1) * 128]
            c0 = 128 * i
            ps = psum_pool.tile([128, BAND], F32)
            for jj in range(4):
                nc.tensor.matmul(
                    ps[:, jj * 512:(jj + 1) * 512], lh,
                    rhs_sb[:, c0 + jj * 512:c0 + (jj + 1) * 512],
                    start=True, stop=True,
                )
            # 3D so the row-min reduce emits a multi-element packed output,
            # a condition for the DVE 2x performance mode
            cst = cast_pool.tile([128, 4, 512], F16)
            nc.scalar.copy(cst[:], ps[:])
            # column-min accumulate (serial across tiles; one op per ~2us
            # tile period so the chain never binds)
            vmin(colacc[:, c0:c0 + BAND], cst[:], colacc[:, c0:c0 + BAND])
            # row-min: two tail variants alternate so one trace measures both
            if i % 2 == 0:
                s4 = rfold_pool.tile([128, 4], F16)
                nc.vector.tensor_reduce(
                    s4[:], cst[:], axis=mybir.AxisListType.X, op=MIN,
                )
                nc.vector.tensor_reduce(
                    rowminb[:, i:i + 1], s4[:], axis=mybir.AxisListType.X,
                    op=MIN,
                )
            else:
                rfold = rfold_pool.tile([128, 2, 512], F16)
                vmin(rfold[:], cst[:, 0:2], cst[:, 2:4])
                vmin(rfold[:, 0], rfold[:, 0], rfold[:, 1])
                nc.vector.tensor_reduce(
                    rowminb[:, i:i + 1], rfold[:, 0],
                    axis=mybir.AxisListType.X, op=MIN,
                )
            for k in range(nchunks):
                if last_touch[k] == i:
                    nc.sync.dma_start(
                        o_cm[:, 1024 * k:1024 * (k + 1)],
                        colacc[:, 1024 * k:1024 * (k + 1)],
                    )
        nc.sync.dma_start(o_rm, rowminb[:])
    if split_waits:
        split_multi_waits(nc)  # CoreSim can't model the injected waits
    return nc


_program_cache = {}


def _get_program():
    if "p" not in _program_cache:
        _program_cache["p"] = build_program()
    return _program_cache["p"]


def enable_profiling():
    """Wire up the NTFF profiling hook (the image's antenv lacks
    antenv.axon_hooks) and neuter the credential-requiring artifact upload.
    Needed only when tracing (BASS_TRACE=1); harmless otherwise."""
    import types
    import antenv
    import concourse.bass_utils as _bu

    if "antenv.axon_hooks" not in sys.modules:
        hooks = types.ModuleType("antenv.axon_hooks")
        hooks._h = None
        hooks.set_axon_ntff_profile_hook = lambda h: setattr(hooks, "_h", h)
        hooks.get_axon_ntff_profile_hook = lambda: hooks._h
        sys.modules["antenv.axon_hooks"] = hooks
        antenv.axon_hooks = hooks
        try:
            from trn_agent_boot.trn_boot import _ntff_profile_via_ctypes

            hooks.set_axon_ntff_profile_hook(
                _ntff_profile_via_ctypes("/opt/axon/libaxon_pjrt.so")
            )
        except Exception:
            pass
    _bu.upload_artifacts = lambda tmpdir: f"local:{tmpdir}"


if os.environ.get("BASS_TRACE"):
    try:
        enable_profiling()
    except Exception:
        pass


def _split3(x):
    x1 = x.astype(BF16NP)
    r = x - x1.astype(np.float32)
    x2 = r.astype(BF16NP)
    x3 = (r - x2.astype(np.float32)).astype(BF16NP)
    return x1, x2, x3


def _aug_dekker(t, s, s_sq):
    """K=24 bf16 3-way-split augmentation. Each fp32 value a = a1+a2+a3 in
    bf16 parts; products kept to O(2^-27): a1b1, a1b2, a2b1, a1b3, a3b1,
    a2b2. PE cost is free-dim cycles only, so K=24 runs as fast as K=5."""
    rows, cols = t.shape[0], s.shape[0]
    t1, t2, t3 = _split3(t)
    s1, s2, s3 = _split3(-2.0 * s)
    n1, n2, n3 = _split3((t * t).sum(axis=1))
    m1, m2, m3 = _split3(s_sq)
    one = np.ones((), BF16NP)
    lhsT = np.empty((24, rows), BF16NP)
    for j, part in enumerate((t1, t1, t2, t1, t3, t2)):
        lhsT[3 * j:3 * j + 3] = part.T
    lhsT[18] = n1
    lhsT[19] = n2
    lhsT[20] = n3
    lhsT[21:24] = one
    rhs = np.empty((24, cols), BF16NP)
    for j, part in enumerate((s1, s2, s1, s3, s1, s2)):
        rhs[3 * j:3 * j + 3] = part.T
    rhs[18:21] = one
    rhs[21] = m1
    rhs[22] = m2
    rhs[23] = m3
    return lhsT, rhs


def _prep(template, source):
    """Sort per batch by z; build per-core shifted+padded source windows."""
    template = np.asarray(template, dtype=np.float32)
    source = np.asarray(source, dtype=np.float32)
    tb_all, sb_all, in_maps = [], [], []
    for b in range(B):
        tb = template[b][np.argsort(template[b][:, 2], kind="stable")]
        sb = source[b][np.argsort(source[b][:, 2], kind="stable")]
        tb_all.append(tb)
        sb_all.append(sb)
    for c in range(NCORES):
        b, h = c // 2, c % 2
        tb, sb = tb_all[b], sb_all[b]
        t = tb[h * HALF:(h + 1) * HALF]
        lo = HALF * h - W
        idx = np.arange(lo, lo + VCOLS)
        valid = (idx >= 0) & (idx < M)
        s = np.where(valid[:, None], sb[np.clip(idx, 0, M - 1)], 0.0)
        s_sq = np.where(valid, (s * s).sum(axis=1), SENT_SQ).astype(np.float32)
        lhsT, rhs = _aug_dekker(t, s.astype(np.float32), s_sq)
        in_maps.append(
            {"lhsT_aug": np.ascontiguousarray(lhsT),
             "rhs_aug": np.ascontiguousarray(rhs)}
        )
    return tb_all, sb_all, in_maps


last_results = None  # BassKernelResults of the most recent kernel() call


def kernel(template, source):
    global last_results
    nc = _get_program()
    tb_all, sb_all, in_maps = _prep(template, source)
    res = run_bass_kernel_spmd(nc, in_maps, list(range(NCORES)))
    last_results = res

    per_batch = np.zeros(B, dtype=np.float64)
    for b in range(B):
        tb = tb_all[b].astype(np.float64)
        sb = sb_all[b].astype(np.float64)
        ztb, zsb = tb[:, 2], sb[:, 2]

        rowmin = np.empty(N, dtype=np.float64)
        colmin = np.full(M, np.inf)
        for h in range(2):
            r = res.results[2 * b + h]
            rm = r["out_rowmin"].astype(np.float64)  # [128, 32]
            # column j of rm = per-row minima of tile j (rows = partitions)
            rowmin[h * HALF:(h + 1) * HALF] = rm.T.reshape(-1)
            cm = r["out_colmin"].astype(np.float64)[:].min(axis=0)  # [VCOLS]
            lo = HALF * h - W
            idx = np.arange(lo, lo + VCOLS)
            valid = (idx >= 0) & (idx < M)
            np.minimum.at(colmin, np.clip(idx, 0, M - 1),
                          np.where(valid, cm, np.inf))

        # --- exactness patch: rows whose banded min could be beaten by an
        # out-of-band source point (d_outside >= z_gap^2) ---
        g = np.arange(N) // 128
        band_lo = HALF * (g // ROW_TILES) - W + 128 * (g % ROW_TILES)
        band_hi = band_lo + BAND
        gap_lo = np.where(band_lo > 0,
                          np.abs(ztb - zsb[np.clip(band_lo - 1, 0, M - 1)]),
                          np.inf)
        gap_hi = np.where(band_hi < M,
                          np.abs(zsb[np.clip(band_hi, 0, M - 1)] - ztb),
                          np.inf)
        gap2 = np.minimum(gap_lo, gap_hi) ** 2
        flag_r = np.where(rowmin > gap2 * 0.95 - 1e-4)[0]
        if len(flag_r):
            d = ((tb[flag_r][:, None, :] - sb[None, :, :]) ** 2).sum(-1)
            rowmin[flag_r] = d.min(axis=1)

        # --- exactness patch: columns (symmetric) ---
        tile_lo = HALF * (np.arange(2 * ROW_TILES) // ROW_TILES) - W \
            + 128 * (np.arange(2 * ROW_TILES) % ROW_TILES)
        v = np.arange(M)
        cov = (v[None, :] >= tile_lo[:, None]) & \
              (v[None, :] < tile_lo[:, None] + BAND)  # [64, M]
        n_lo = np.argmax(cov, axis=0) * 128
        n_hi = (len(cov) - 1 - np.argmax(cov[::-1], axis=0)) * 128 + 128
        zg_lo = np.where(n_lo > 0,
                         np.abs(zsb - ztb[np.clip(n_lo - 1, 0, N - 1)]),
                         np.inf)
        zg_hi = np.where(n_hi < N,
                         np.abs(ztb[np.clip(n_hi, 0, N - 1)] - zsb),
                         np.inf)
        gap2c = np.minimum(zg_lo, zg_hi) ** 2
        flag_c = np.where(colmin > gap2c * 0.95 - 1e-4)[0]
        if len(flag_c):
            d = ((tb[None, :, :] - sb[flag_c][:, None, :]) ** 2).sum(-1)
            colmin[flag_c] = d.min(axis=1)

        per_batch[b] = rowmin.mean() + colmin.mean()
    return np.float32(per_batch.mean())


# revision 8
# speedup vs baseline: 4.5886x; 1.6790x over previous
"""Chamfer distance loss kernel for Trainium2 (8 NeuronCores).

Problem: template/source (4, 8192, 3) f32. For each batch b:
  d[n,m] = |t_n|^2 - 2 t_n.s_m + |s_m|^2
  loss_b = mean_n min_m d + mean_m min_n d ; output = mean_b loss_b (scalar).

Strategy: z-sorted banding. Both point sets are sorted by z on the host
(per batch). A 128-row template tile then only needs a contiguous band
of ~1024 z-sorted source columns: nearest neighbors are z-close for all
but a handful of radial outliers. This cuts the distance-matrix volume
8x (32 tiles x 1024 cols per core instead of 32 x 8192).

The banding is made EXACT by a host-side patch: a banded row-min can
only be wrong if the true nearest neighbor lies outside the band, which
implies d >= (z-gap to the band edge)^2. Rows/cols whose banded min
exceeds that bound are recomputed exactly in numpy (a few hundred per
batch, milliseconds). Means are permutation-invariant so the sort needs
no undoing.

Sharding: core c = (batch c//2, sorted-template-half c%2); each core's
source input is a pre-shifted 5120-rank window of the sorted source
(rank range [4096h-448, 4096h+4672)) padded with far-away sentinel
points, which keeps one SPMD program valid for both halves: tile i
always scans virtual columns [128i, 128i+1024).

Per core, each [128 x 512] distance tile is produced in PSUM by ONE
augmented matmul: d = |t|^2 - 2 t.s + |s|^2 as a K=24 contraction of
bf16 3-way value splits (fp32-grade distances at 1 cycle/row). ScalarE
casts each 1024-col PSUM group to fp16 once; VectorE does the row-min
reduce and the column-min accumulate in fp16 2x packed mode. Column
minima ship to HBM in 1024-col chunks as soon as their last touching
tile has passed, overlapping output DMA with compute.
"""
import os
import sys

sys.path.insert(0, "/opt/trn_rl_repo")

from contextlib import ExitStack

import numpy as np

import concourse.bass as bass
import concourse.tile as tile
from concourse import mybir
from concourse.bass_utils import run_bass_kernel_spmd

# ---------------------------------------------------------------------------
# The walrus build in this container rejects instructions carrying more than
# one sync-wait command. After Tile scheduling, split any multi-wait
# instruction: keep the first wait on it and hoist the rest onto standalone
# EventSemaphore instructions inserted just before it (same engine, so
# per-engine program order makes the waits execute first).
import bass_rust as _br


def split_multi_waits(nc):
    n_new = 0
    for fn in nc.m.functions:
        for blk in fn.blocks:
            insts = list(blk.instructions)
            out = []
            changed = False
            for inst in insts:
                si = inst.sync_info
                waits = list(si.on_wait) if si is not None and si.on_wait else []
                if len(waits) > 1:
                    for w in waits[:-1]:
                        ev = _br.InstEventSemaphore(
                            name=f"I-waitsplit-{n_new}", ins=[], outs=[]
                        )
                        n_new += 1
                        ev.engine = inst.engine
                        ev.sync_info = _br.SyncInfo(on_wait=[w], on_update=[])
                        out.append(ev)
                    si.on_wait = [waits[-1]]
                    changed = True
                out.append(inst)
            if changed:
                blk.instructions = out
# ---------------------------------------------------------------------------

import ml_dtypes

F32 = mybir.dt.float32
F16 = mybir.dt.float16
BF16 = mybir.dt.bfloat16
MIN = mybir.AluOpType.min
BF16NP = ml_dtypes.bfloat16

B, N, M, D = 4, 8192, 8192, 3
HALF = N // 2       # template rows per core
NCORES = 8
K = 24              # dekker-split contraction depth
W = 448             # band half-width in source ranks
BAND = 1024         # columns per tile band (= 128 + 2*W)
VCOLS = HALF + 2 * W + 128  # virtual source columns per core = 6144
ROW_TILES = HALF // 128     # 32
BIG = 60000.0       # fp16-safe "+inf" for min accumulators
SENT_SQ = 30000.0   # sentinel |s|^2: d at padded columns never wins a min


def build_program(split_waits=True):
    nc = bass.Bass("TRN2", target_bir_lowering=False, debug=False)
    lhsT = nc.dram_tensor("lhsT_aug", [K, HALF], BF16, kind="ExternalInput").ap()
    rhs = nc.dram_tensor("rhs_aug", [K, VCOLS], BF16, kind="ExternalInput").ap()
    o_rm = nc.dram_tensor(
        "out_rowmin", [128, ROW_TILES], F16, kind="ExternalOutput"
    ).ap()
    o_cm = nc.dram_tensor("out_colmin", [128, VCOLS], F16, kind="ExternalOutput").ap()

    def vmin(out_ap, a_ap, b_ap):
        nc.vector.tensor_tensor(out_ap, a_ap, b_ap, op=MIN)

    # last tile index that touches each 1024-col colacc chunk: ship chunks
    # as they become final so output DMA overlaps compute
    nchunks = VCOLS // 1024
    last_touch = [
        max(i for i in range(ROW_TILES)
            if 128 * i < 1024 * (k + 1) and 128 * i + BAND > 1024 * k)
        for k in range(nchunks)
    ]

    with tile.TileContext(nc) as tc, ExitStack() as ctx:
        consts = ctx.enter_context(tc.tile_pool(name="consts", bufs=1))
        psum_pool = ctx.enter_context(tc.tile_pool(name="psum", bufs=4, space="PSUM"))
        cast_pool = ctx.enter_context(tc.tile_pool(name="cast", bufs=4))
        rfold_pool = ctx.enter_context(tc.tile_pool(name="rfold", bufs=2))
        accs = ctx.enter_context(tc.tile_pool(name="accs", bufs=1))

        # warm the ACT function-table (its ~2.7us load overlaps input DMA)
        warm = consts.tile([1, 16], F16)
        nc.vector.memset(warm[:], 0.0)
        nc.scalar.copy(warm[:], warm[:])

        lhsT_sb = consts.tile([K, HALF], BF16)
        rhs_sb = consts.tile([K, VCOLS], BF16)
        # first tile's operands land first so matmuls start early
        nc.sync.dma_start(lhsT_sb[:, 0:128], lhsT[:, 0:128])
        for q in range(VCOLS // 1024):
            nc.sync.dma_start(
                rhs_sb[:, q * 1024:(q + 1) * 1024],
                rhs[:, q * 1024:(q + 1) * 1024],
            )
        nc.sync.dma_start(lhsT_sb[:, 128:HALF], lhsT[:, 128:HALF])

        colacc = accs.tile([128, VCOLS], F16)
        # GpSimd is otherwise idle; keep the init off the busy VectorE
        nc.gpsimd.memset(colacc[:], BIG)
        rowminb = accs.tile([128, ROW_TILES], F16)

        nmm = BAND // 512
        for i in range(ROW_TILES):
            lh = lhsT_sb[:, i * 128:(i + 1) * 128]
            c0 = 128 * i
            ps = psum_pool.tile([128, BAND], F32)
            for jj in range(nmm):
                nc.tensor.matmul(
                    ps[:, jj * 512:(jj + 1) * 512], lh,
                    rhs_sb[:, c0 + jj * 512:c0 + (jj + 1) * 512],
                    start=True, stop=True,
                )
            cst = cast_pool.tile([128, nmm, 512], F16)
            nc.scalar.copy(cst[:], ps[:])
            # column-min accumulate (serial across tiles; one op per tile
            # period so the chain never binds)
            vmin(colacc[:, c0:c0 + BAND], cst[:], colacc[:, c0:c0 + BAND])
            # row-min: fold halves with a 2x tensor_tensor, then reduce
            rfold = rfold_pool.tile([128, 512], F16)
            vmin(rfold[:], cst[:, 0], cst[:, 1])
            nc.vector.tensor_reduce(
                rowminb[:, i:i + 1], rfold[:],
                axis=mybir.AxisListType.X, op=MIN,
            )
            for k in range(nchunks):
                if last_touch[k] == i:
                    nc.sync.dma_start(
                        o_cm[:, 1024 * k:1024 * (k + 1)],
                        colacc[:, 1024 * k:1024 * (k + 1)],
                    )
        nc.sync.dma_start(o_rm, rowminb[:])
    if split_waits:
        split_multi_waits(nc)  # CoreSim can't model the injected waits
    return nc


_program_cache = {}


def _get_program():
    if "p" not in _program_cache:
        _program_cache["p"] = build_program()
    return _program_cache["p"]


def enable_profiling():
    """Wire up the NTFF profiling hook (the image's antenv lacks
    antenv.axon_hooks) and neuter the credential-requiring artifact upload.
    Needed only when tracing (BASS_TRACE=1); harmless otherwise."""
    import types
    import antenv
    import concourse.bass_utils as _bu

    if "antenv.axon_hooks" not in sys.modules:
        hooks = types.ModuleType("antenv.axon_hooks")
        hooks._h = None
        hooks.set_axon_ntff_profile_hook = lambda h: setattr(hooks, "_h", h)
        hooks.get_axon_ntff_profile_hook = lambda: hooks._h
        sys.modules["antenv.axon_hooks"] = hooks
        antenv.axon_hooks = hooks
        try:
            from trn_agent_boot.trn_boot import _ntff_profile_via_ctypes

            hooks.set_axon_ntff_profile_hook(
                _ntff_profile_via_ctypes("/opt/axon/libaxon_pjrt.so")
            )
        except Exception:
            pass
    _bu.upload_artifacts = lambda tmpdir: f"local:{tmpdir}"


if os.environ.get("BASS_TRACE"):
    try:
        enable_profiling()
    except Exception:
        pass


def _split3(x):
    x1 = x.astype(BF16NP)
    r = x - x1.astype(np.float32)
    x2 = r.astype(BF16NP)
    x3 = (r - x2.astype(np.float32)).astype(BF16NP)
    return x1, x2, x3


def _aug_dekker(t, s, s_sq):
    """K=24 bf16 3-way-split augmentation. Each fp32 value a = a1+a2+a3 in
    bf16 parts; products kept to O(2^-27): a1b1, a1b2, a2b1, a1b3, a3b1,
    a2b2. PE cost is free-dim cycles only, so K=24 runs as fast as K=5."""
    rows, cols = t.shape[0], s.shape[0]
    t1, t2, t3 = _split3(t)
    s1, s2, s3 = _split3(-2.0 * s)
    n1, n2, n3 = _split3((t * t).sum(axis=1))
    m1, m2, m3 = _split3(s_sq)
    one = np.ones((), BF16NP)
    lhsT = np.empty((24, rows), BF16NP)
    for j, part in enumerate((t1, t1, t2, t1, t3, t2)):
        lhsT[3 * j:3 * j + 3] = part.T
    lhsT[18] = n1
    lhsT[19] = n2
    lhsT[20] = n3
    lhsT[21:24] = one
    rhs = np.empty((24, cols), BF16NP)
    for j, part in enumerate((s1, s2, s1, s3, s1, s2)):
        rhs[3 * j:3 * j + 3] = part.T
    rhs[18:21] = one
    rhs[21] = m1
    rhs[22] = m2
    rhs[23] = m3
    return lhsT, rhs


def _prep(template, source):
    """Sort per batch by z; build per-core shifted+padded source windows."""
    template = np.asarray(template, dtype=np.float32)
    source = np.asarray(source, dtype=np.float32)
    tb_all, sb_all, in_maps = [], [], []
    for b in range(B):
        tb = template[b][np.argsort(template[b][:, 2], kind="stable")]
        sb = source[b][np.argsort(source[b][:, 2], kind="stable")]
        tb_all.append(tb)
        sb_all.append(sb)
    for c in range(NCORES):
        b, h = c // 2, c % 2
        tb, sb = tb_all[b], sb_all[b]
        t = tb[h * HALF:(h + 1) * HALF]
        lo = HALF * h - W
        idx = np.arange(lo, lo + VCOLS)
        valid = (idx >= 0) & (idx < M)
        s = np.where(valid[:, None], sb[np.clip(idx, 0, M - 1)], 0.0)
        s_sq = np.where(valid, (s * s).sum(axis=1), SENT_SQ).astype(np.float32)
        lhsT, rhs = _aug_dekker(t, s.astype(np.float32), s_sq)
        in_maps.append(
            {"lhsT_aug": np.ascontiguousarray(lhsT),
             "rhs_aug": np.ascontiguousarray(rhs)}
        )
    return tb_all, sb_all, in_maps


last_results = None  # BassKernelResults of the most recent kernel() call


def kernel(template, source):
    global last_results
    nc = _get_program()
    tb_all, sb_all, in_maps = _prep(template, source)
    res = run_bass_kernel_spmd(nc, in_maps, list(range(NCORES)))
    last_results = res

    per_batch = np.zeros(B, dtype=np.float64)
    for b in range(B):
        tb = tb_all[b].astype(np.float64)
        sb = sb_all[b].astype(np.float64)
        ztb, zsb = tb[:, 2], sb[:, 2]

        rowmin = np.empty(N, dtype=np.float64)
        colmin = np.full(M, np.inf)
        for h in range(2):
            r = res.results[2 * b + h]
            rm = r["out_rowmin"].astype(np.float64)  # [128, 32]
            # column j of rm = per-row minima of tile j (rows = partitions)
            rowmin[h * HALF:(h + 1) * HALF] = rm.T.reshape(-1)
            cm = r["out_colmin"].astype(np.float64)[:].min(axis=0)  # [VCOLS]
            lo = HALF * h - W
            idx = np.arange(lo, lo + VCOLS)
            valid = (idx >= 0) & (idx < M)
            np.minimum.at(colmin, np.clip(idx, 0, M - 1),
                          np.where(valid, cm, np.inf))

        # --- exactness patch: rows whose banded min could be beaten by an
        # out-of-band source point (d_outside >= z_gap^2) ---
        g = np.arange(N) // 128
        band_lo = HALF * (g // ROW_TILES) - W + 128 * (g % ROW_TILES)
        band_hi = band_lo + BAND
        gap_lo = np.where(band_lo > 0,
                          np.abs(ztb - zsb[np.clip(band_lo - 1, 0, M - 1)]),
                          np.inf)
        gap_hi = np.where(band_hi < M,
                          np.abs(zsb[np.clip(band_hi, 0, M - 1)] - ztb),
                          np.inf)
        gap2 = np.minimum(gap_lo, gap_hi) ** 2
        flag_r = np.where(rowmin > gap2 * 0.95 - 1e-4)[0]
        if len(flag_r):
            d = ((tb[flag_r][:, None, :] - sb[None, :, :]) ** 2).sum(-1)
            rowmin[flag_r] = d.min(axis=1)

        # --- exactness patch: columns (symmetric) ---
        tile_lo = HALF * (np.arange(2 * ROW_TILES) // ROW_TILES) - W \
            + 128 * (np.arange(2 * ROW_TILES) % ROW_TILES)
        v = np.arange(M)
        cov = (v[None, :] >= tile_lo[:, None]) & \
              (v[None, :] < tile_lo[:, None] + BAND)  # [64, M]
        n_lo = np.argmax(cov, axis=0) * 128
        n_hi = (len(cov) - 1 - np.argmax(cov[::-1], axis=0)) * 128 + 128
        zg_lo = np.where(n_lo > 0,
                         np.abs(zsb - ztb[np.clip(n_lo - 1, 0, N - 1)]),
                         np.inf)
        zg_hi = np.where(n_hi < N,
                         np.abs(ztb[np.clip(n_hi, 0, N - 1)] - zsb),
                         np.inf)
        gap2c = np.minimum(zg_lo, zg_hi) ** 2
        flag_c = np.where(colmin > gap2c * 0.95 - 1e-4)[0]
        if len(flag_c):
            d = ((tb[None, :, :] - sb[flag_c][:, None, :]) ** 2).sum(-1)
            colmin[flag_c] = d.min(axis=1)

        per_batch[b] = rowmin.mean() + colmin.mean()
    return np.float32(per_batch.mean())


# revision 12
# speedup vs baseline: 6.6296x; 1.4448x over previous
"""Chamfer distance loss kernel for Trainium2 (8 NeuronCores).

Problem: template/source (4, 8192, 3) f32. For each batch b:
  d[n,m] = |t_n|^2 - 2 t_n.s_m + |s_m|^2
  loss_b = mean_n min_m d + mean_m min_n d ; output = mean_b loss_b (scalar).

Strategy: z-sorted banding. Both point sets are sorted by z on the host
(per batch). A 128-row template tile then only needs a contiguous band
of ~1024 z-sorted source columns: nearest neighbors are z-close for all
but a handful of radial outliers. This cuts the distance-matrix volume
8x (32 tiles x 1024 cols per core instead of 32 x 8192).

The banding is made EXACT by a host-side patch: a banded row-min can
only be wrong if the true nearest neighbor lies outside the band, which
implies d >= (z-gap to the band edge)^2. Rows/cols whose banded min
exceeds that bound are recomputed exactly in numpy (a few hundred per
batch, milliseconds). Means are permutation-invariant so the sort needs
no undoing.

Sharding: core c = (batch c//2, sorted-template-half c%2); each core's
source input is a pre-shifted 5120-rank window of the sorted source
(rank range [4096h-448, 4096h+4672)) padded with far-away sentinel
points, which keeps one SPMD program valid for both halves: tile i
always scans virtual columns [128i, 128i+1024).

Per core, each [128 x 512] distance tile is produced in PSUM by ONE
augmented matmul: d = |t|^2 - 2 t.s + |s|^2 as a K=24 contraction of
bf16 3-way value splits (fp32-grade distances at 1 cycle/row). ScalarE
casts each 1024-col PSUM group to fp16 once; VectorE does the row-min
reduce and the column-min accumulate in fp16 2x packed mode. Column
minima ship to HBM in 1024-col chunks as soon as their last touching
tile has passed, overlapping output DMA with compute.
"""
import os
import sys

sys.path.insert(0, "/opt/trn_rl_repo")

from contextlib import ExitStack

import numpy as np

import concourse.bass as bass
import concourse.tile as tile
from concourse import mybir
from concourse.bass_utils import run_bass_kernel_spmd

# ---------------------------------------------------------------------------
# The walrus build in this container rejects instructions carrying more than
# one sync-wait command. After Tile scheduling, split any multi-wait
# instruction: keep the first wait on it and hoist the rest onto standalone
# EventSemaphore instructions inserted just before it (same engine, so
# per-engine program order makes the waits execute first).
import bass_rust as _br


def split_multi_waits(nc):
    n_new = 0
    for fn in nc.m.functions:
        for blk in fn.blocks:
            insts = list(blk.instructions)
            out = []
            changed = False
            for inst in insts:
                si = inst.sync_info
                waits = list(si.on_wait) if si is not None and si.on_wait else []
                if len(waits) > 1:
                    for w in waits[:-1]:
                        ev = _br.InstEventSemaphore(
                            name=f"I-waitsplit-{n_new}", ins=[], outs=[]
                        )
                        n_new += 1
                        ev.engine = inst.engine
                        ev.sync_info = _br.SyncInfo(on_wait=[w], on_update=[])
                        out.append(ev)
                    si.on_wait = [waits[-1]]
                    changed = True
                out.append(inst)
            if changed:
                blk.instructions = out
# ---------------------------------------------------------------------------

import ml_dtypes

F32 = mybir.dt.float32
F16 = mybir.dt.float16
BF16 = mybir.dt.bfloat16
MIN = mybir.AluOpType.min
BF16NP = ml_dtypes.bfloat16

B, N, M, D = 4, 8192, 8192, 3
HALF = N // 2       # template rows per core
NCORES = 8
K = 24              # dekker-split contraction depth
W = 192             # band half-width in source ranks
BAND = 512          # columns per tile band (= 128 + 2*W)
VCOLS = HALF + 2 * W + 128  # virtual source columns per core = 6144
ROW_TILES = HALF // 128     # 32
BIG = 60000.0       # fp16-safe "+inf" for min accumulators
SENT_SQ = 30000.0   # sentinel |s|^2: d at padded columns never wins a min


def build_program(split_waits=True):
    nc = bass.Bass("TRN2", target_bir_lowering=False, debug=False)
    lhsT = nc.dram_tensor("lhsT_aug", [K, HALF], BF16, kind="ExternalInput").ap()
    rhs = nc.dram_tensor("rhs_aug", [K, VCOLS], BF16, kind="ExternalInput").ap()
    o_rm = nc.dram_tensor(
        "out_rowmin", [128, ROW_TILES], F16, kind="ExternalOutput"
    ).ap()
    o_cm = nc.dram_tensor("out_colmin", [128, VCOLS], F16, kind="ExternalOutput").ap()

    def vmin(out_ap, a_ap, b_ap):
        nc.vector.tensor_tensor(out_ap, a_ap, b_ap, op=MIN)

    # last tile index that touches each 512-col colacc chunk: ship chunks
    # as they become final so output DMA overlaps compute
    CHUNK = 512
    nchunks = VCOLS // CHUNK
    last_touch = [
        max(i for i in range(ROW_TILES)
            if 128 * i < CHUNK * (k + 1) and 128 * i + BAND > CHUNK * k)
        for k in range(nchunks)
    ]

    with tile.TileContext(nc) as tc, ExitStack() as ctx:
        consts = ctx.enter_context(tc.tile_pool(name="consts", bufs=1))
        psum_pool = ctx.enter_context(tc.tile_pool(name="psum", bufs=4, space="PSUM"))
        cast_pool = ctx.enter_context(tc.tile_pool(name="cast", bufs=4))
        rfold_pool = ctx.enter_context(tc.tile_pool(name="rfold", bufs=2))
        accs = ctx.enter_context(tc.tile_pool(name="accs", bufs=1))

        # warm the ACT function-table (its ~2.7us load overlaps input DMA)
        warm = consts.tile([1, 16], F16)
        nc.vector.memset(warm[:], 0.0)
        nc.scalar.copy(warm[:], warm[:])

        lhsT_sb = consts.tile([K, HALF], BF16)
        rhs_sb = consts.tile([K, VCOLS], BF16)
        # few, large input transfers: SWDGE descriptor generation is ~600ns
        # apiece and serializes ahead of the first matmul
        nc.sync.dma_start(lhsT_sb[:, 0:128], lhsT[:, 0:128])
        nc.sync.dma_start(rhs_sb[:, 0:1024], rhs[:, 0:1024])
        nc.sync.dma_start(rhs_sb[:, 1024:VCOLS], rhs[:, 1024:VCOLS])
        nc.sync.dma_start(lhsT_sb[:, 128:HALF], lhsT[:, 128:HALF])

        colacc = accs.tile([128, VCOLS], F16)
        # GpSimd is otherwise idle; keep the init off the busy VectorE
        nc.gpsimd.memset(colacc[:], BIG)
        rowminb = accs.tile([128, ROW_TILES], F16)

        nmm = BAND // 512
        for i in range(ROW_TILES):
            lh = lhsT_sb[:, i * 128:(i + 1) * 128]
            c0 = 128 * i
            ps = psum_pool.tile([128, BAND], F32)
            for jj in range(nmm):
                nc.tensor.matmul(
                    ps[:, jj * 512:(jj + 1) * 512], lh,
                    rhs_sb[:, c0 + jj * 512:c0 + (jj + 1) * 512],
                    start=True, stop=True,
                )
            cst = cast_pool.tile([128, nmm, 512], F16)
            nc.scalar.copy(cst[:], ps[:])
            # column-min accumulate (serial across tiles; one op per tile
            # period so the chain never binds)
            vmin(colacc[:, c0:c0 + BAND], cst[:], colacc[:, c0:c0 + BAND])
            # row-min
            if nmm == 1:
                nc.vector.tensor_reduce(
                    rowminb[:, i:i + 1], cst[:, 0],
                    axis=mybir.AxisListType.X, op=MIN,
                )
            else:
                rfold = rfold_pool.tile([128, 512], F16)
                vmin(rfold[:], cst[:, 0], cst[:, 1])
                nc.vector.tensor_reduce(
                    rowminb[:, i:i + 1], rfold[:],
                    axis=mybir.AxisListType.X, op=MIN,
                )
            for k in range(nchunks):
                if last_touch[k] == i:
                    nc.sync.dma_start(
                        o_cm[:, CHUNK * k:CHUNK * (k + 1)],
                        colacc[:, CHUNK * k:CHUNK * (k + 1)],
                    )
        nc.sync.dma_start(o_rm, rowminb[:])
    if split_waits:
        split_multi_waits(nc)  # CoreSim can't model the injected waits
    return nc


_program_cache = {}


def _get_program():
    if "p" not in _program_cache:
        _program_cache["p"] = build_program()
    return _program_cache["p"]


def enable_profiling():
    """Wire up the NTFF profiling hook (the image's antenv lacks
    antenv.axon_hooks) and neuter the credential-requiring artifact upload.
    Needed only when tracing (BASS_TRACE=1); harmless otherwise."""
    import types
    import antenv
    import concourse.bass_utils as _bu

    if "antenv.axon_hooks" not in sys.modules:
        hooks = types.ModuleType("antenv.axon_hooks")
        hooks._h = None
        hooks.set_axon_ntff_profile_hook = lambda h: setattr(hooks, "_h", h)
        hooks.get_axon_ntff_profile_hook = lambda: hooks._h
        sys.modules["antenv.axon_hooks"] = hooks
        antenv.axon_hooks = hooks
        try:
            from trn_agent_boot.trn_boot import _ntff_profile_via_ctypes

            hooks.set_axon_ntff_profile_hook(
                _ntff_profile_via_ctypes("/opt/axon/libaxon_pjrt.so")
            )
        except Exception:
            pass
    _bu.upload_artifacts = lambda tmpdir: f"local:{tmpdir}"


if os.environ.get("BASS_TRACE"):
    try:
        enable_profiling()
    except Exception:
        pass


def _split3(x):
    x1 = x.astype(BF16NP)
    r = x - x1.astype(np.float32)
    x2 = r.astype(BF16NP)
    x3 = (r - x2.astype(np.float32)).astype(BF16NP)
    return x1, x2, x3


def _aug_dekker(t, s, s_sq):
    """K=24 bf16 3-way-split augmentation. Each fp32 value a = a1+a2+a3 in
    bf16 parts; products kept to O(2^-27): a1b1, a1b2, a2b1, a1b3, a3b1,
    a2b2. PE cost is free-dim cycles only, so K=24 runs as fast as K=5."""
    rows, cols = t.shape[0], s.shape[0]
    t1, t2, t3 = _split3(t)
    s1, s2, s3 = _split3(-2.0 * s)
    n1, n2, n3 = _split3((t * t).sum(axis=1))
    m1, m2, m3 = _split3(s_sq)
    one = np.ones((), BF16NP)
    lhsT = np.empty((24, rows), BF16NP)
    for j, part in enumerate((t1, t1, t2, t1, t3, t2)):
        lhsT[3 * j:3 * j + 3] = part.T
    lhsT[18] = n1
    lhsT[19] = n2
    lhsT[20] = n3
    lhsT[21:24] = one
    rhs = np.empty((24, cols), BF16NP)
    for j, part in enumerate((s1, s2, s1, s3, s1, s2)):
        rhs[3 * j:3 * j + 3] = part.T
    rhs[18:21] = one
    rhs[21] = m1
    rhs[22] = m2
    rhs[23] = m3
    return lhsT, rhs


def _prep(template, source):
    """Sort per batch by z; build per-core shifted+padded source windows."""
    template = np.asarray(template, dtype=np.float32)
    source = np.asarray(source, dtype=np.float32)
    tb_all, sb_all, in_maps = [], [], []
    for b in range(B):
        tb = template[b][np.argsort(template[b][:, 2], kind="stable")]
        sb = source[b][np.argsort(source[b][:, 2], kind="stable")]
        tb_all.append(tb)
        sb_all.append(sb)
    for c in range(NCORES):
        b, h = c // 2, c % 2
        tb, sb = tb_all[b], sb_all[b]
        t = tb[h * HALF:(h + 1) * HALF]
        lo = HALF * h - W
        idx = np.arange(lo, lo + VCOLS)
        valid = (idx >= 0) & (idx < M)
        s = np.where(valid[:, None], sb[np.clip(idx, 0, M - 1)], 0.0)
        s_sq = np.where(valid, (s * s).sum(axis=1), SENT_SQ).astype(np.float32)
        lhsT, rhs = _aug_dekker(t, s.astype(np.float32), s_sq)
        in_maps.append(
            {"lhsT_aug": np.ascontiguousarray(lhsT),
             "rhs_aug": np.ascontiguousarray(rhs)}
        )
    return tb_all, sb_all, in_maps


last_results = None  # BassKernelResults of the most recent kernel() call


def kernel(template, source):
    global last_results
    nc = _get_program()
    tb_all, sb_all, in_maps = _prep(template, source)
    res = run_bass_kernel_spmd(nc, in_maps, list(range(NCORES)))
    last_results = res

    per_batch = np.zeros(B, dtype=np.float64)
    for b in range(B):
        tb = tb_all[b].astype(np.float64)
        sb = sb_all[b].astype(np.float64)
        ztb, zsb = tb[:, 2], sb[:, 2]

        rowmin = np.empty(N, dtype=np.float64)
        colmin = np.full(M, np.inf)
        for h in range(2):
            r = res.results[2 * b + h]
            rm = r["out_rowmin"].astype(np.float64)  # [128, 32]
            # column j of rm = per-row minima of tile j (rows = partitions)
            rowmin[h * HALF:(h + 1) * HALF] = rm.T.reshape(-1)
            cm = r["out_colmin"].astype(np.float64)[:].min(axis=0)  # [VCOLS]
            lo = HALF * h - W
            idx = np.arange(lo, lo + VCOLS)
            valid = (idx >= 0) & (idx < M)
            np.minimum.at(colmin, np.clip(idx, 0, M - 1),
                          np.where(valid, cm, np.inf))

        # --- exactness patch: rows whose banded min could be beaten by an
        # out-of-band source point (d_outside >= z_gap^2) ---
        g = np.arange(N) // 128
        band_lo = HALF * (g // ROW_TILES) - W + 128 * (g % ROW_TILES)
        band_hi = band_lo + BAND
        gap_lo = np.where(band_lo > 0,
                          np.abs(ztb - zsb[np.clip(band_lo - 1, 0, M - 1)]),
                          np.inf)
        gap_hi = np.where(band_hi < M,
                          np.abs(zsb[np.clip(band_hi, 0, M - 1)] - ztb),
                          np.inf)
        gap2 = np.minimum(gap_lo, gap_hi) ** 2
        flag_r = np.where(rowmin > gap2 * 0.95 - 1e-4)[0]
        if len(flag_r):
            d = ((tb[flag_r][:, None, :] - sb[None, :, :]) ** 2).sum(-1)
            rowmin[flag_r] = d.min(axis=1)

        # --- exactness patch: columns (symmetric) ---
        tile_lo = HALF * (np.arange(2 * ROW_TILES) // ROW_TILES) - W \
            + 128 * (np.arange(2 * ROW_TILES) % ROW_TILES)
        v = np.arange(M)
        cov = (v[None, :] >= tile_lo[:, None]) & \
              (v[None, :] < tile_lo[:, None] + BAND)  # [64, M]
        n_lo = np.argmax(cov, axis=0) * 128
        n_hi = (len(cov) - 1 - np.argmax(cov[::-1], axis=0)) * 128 + 128
        zg_lo = np.where(n_lo > 0,
                         np.abs(zsb - ztb[np.clip(n_lo - 1, 0, N - 1)]),
                         np.inf)
        zg_hi = np.where(n_hi < N,
                         np.abs(ztb[np.clip(n_hi, 0, N - 1)] - zsb),
                         np.inf)
        gap2c = np.minimum(zg_lo, zg_hi) ** 2
        flag_c = np.where(colmin > gap2c * 0.95 - 1e-4)[0]
        if len(flag_c):
            d = ((tb[None, :, :] - sb[flag_c][:, None, :]) ** 2).sum(-1)
            colmin[flag_c] = d.min(axis=1)

        per_batch[b] = rowmin.mean() + colmin.mean()
    return np.float32(per_batch.mean())


# revision 14
# speedup vs baseline: 8.7685x; 1.3226x over previous
"""Chamfer distance loss kernel for Trainium2 (8 NeuronCores).

Problem: template/source (4, 8192, 3) f32. For each batch b:
  d[n,m] = |t_n|^2 - 2 t_n.s_m + |s_m|^2
  loss_b = mean_n min_m d + mean_m min_n d ; output = mean_b loss_b (scalar).

Strategy: z-sorted banding. Both point sets are sorted by z on the host
(per batch). A 128-row template tile then only needs a contiguous band
of ~1024 z-sorted source columns: nearest neighbors are z-close for all
but a handful of radial outliers. This cuts the distance-matrix volume
8x (32 tiles x 1024 cols per core instead of 32 x 8192).

The banding is made EXACT by a host-side patch: a banded row-min can
only be wrong if the true nearest neighbor lies outside the band, which
implies d >= (z-gap to the band edge)^2. Rows/cols whose banded min
exceeds that bound are recomputed exactly in numpy (a few hundred per
batch, milliseconds). Means are permutation-invariant so the sort needs
no undoing.

Sharding: core c = (batch c//2, sorted-template-half c%2); each core's
source input is a pre-shifted 5120-rank window of the sorted source
(rank range [4096h-448, 4096h+4672)) padded with far-away sentinel
points, which keeps one SPMD program valid for both halves: tile i
always scans virtual columns [128i, 128i+1024).

Per core, each [128 x 512] distance tile is produced in PSUM by ONE
augmented matmul: d = |t|^2 - 2 t.s + |s|^2 as a K=24 contraction of
bf16 3-way value splits (fp32-grade distances at 1 cycle/row). ScalarE
casts each 1024-col PSUM group to fp16 once; VectorE does the row-min
reduce and the column-min accumulate in fp16 2x packed mode. Column
minima ship to HBM in 1024-col chunks as soon as their last touching
tile has passed, overlapping output DMA with compute.
"""
import os
import sys

sys.path.insert(0, "/opt/trn_rl_repo")

from contextlib import ExitStack

import numpy as np

import concourse.bass as bass
import concourse.tile as tile
from concourse import mybir
from concourse.bass_utils import run_bass_kernel_spmd

# ---------------------------------------------------------------------------
# The walrus build in this container rejects instructions carrying more than
# one sync-wait command. After Tile scheduling, split any multi-wait
# instruction: keep the first wait on it and hoist the rest onto standalone
# EventSemaphore instructions inserted just before it (same engine, so
# per-engine program order makes the waits execute first).
import bass_rust as _br


def split_multi_waits(nc):
    n_new = 0
    for fn in nc.m.functions:
        for blk in fn.blocks:
            insts = list(blk.instructions)
            out = []
            changed = False
            for inst in insts:
                si = inst.sync_info
                waits = list(si.on_wait) if si is not None and si.on_wait else []
                if len(waits) > 1:
                    for w in waits[:-1]:
                        ev = _br.InstEventSemaphore(
                            name=f"I-waitsplit-{n_new}", ins=[], outs=[]
                        )
                        n_new += 1
                        ev.engine = inst.engine
                        ev.sync_info = _br.SyncInfo(on_wait=[w], on_update=[])
                        out.append(ev)
                    si.on_wait = [waits[-1]]
                    changed = True
                out.append(inst)
            if changed:
                blk.instructions = out
# ---------------------------------------------------------------------------

import ml_dtypes

F32 = mybir.dt.float32
F16 = mybir.dt.float16
BF16 = mybir.dt.bfloat16
MIN = mybir.AluOpType.min
BF16NP = ml_dtypes.bfloat16

B, N, M, D = 4, 8192, 8192, 3
HALF = N // 2       # template rows per core
NCORES = 8
K = 24              # dekker-split contraction depth
W = 192             # band half-width in source ranks
BAND = 512          # columns per tile band (= 128 + 2*W)
VCOLS = HALF + 2 * W + 128  # virtual source columns per core = 6144
ROW_TILES = HALF // 128     # 32
BIG = 60000.0       # fp16-safe "+inf" for min accumulators
SENT_SQ = 30000.0   # sentinel |s|^2: d at padded columns never wins a min


TPG = 4                      # row tiles per PSUM group (amortizes cast/reduce)
NGRP = ROW_TILES // TPG      # 8 super-tiles


def build_program(split_waits=True):
    nc = bass.Bass("TRN2", target_bir_lowering=False, debug=False)
    lhsT = nc.dram_tensor("lhsT_aug", [K, HALF], BF16, kind="ExternalInput").ap()
    rhs = nc.dram_tensor("rhs_aug", [K, VCOLS], BF16, kind="ExternalInput").ap()
    o_rm = nc.dram_tensor(
        "out_rowmin", [128, ROW_TILES], F16, kind="ExternalOutput"
    ).ap()
    # the raw banded fp16 distance blocks; the host derives column minima
    # from them (each column is covered by only ~4 tiles)
    o_ct = nc.dram_tensor(
        "out_cst", [NGRP, 128, TPG * 512], F16, kind="ExternalOutput"
    ).ap()

    with tile.TileContext(nc) as tc, ExitStack() as ctx:
        consts = ctx.enter_context(tc.tile_pool(name="consts", bufs=1))
        psum_pool = ctx.enter_context(tc.tile_pool(name="psum", bufs=2, space="PSUM"))
        cast_pool = ctx.enter_context(tc.tile_pool(name="cast", bufs=3))
        accs = ctx.enter_context(tc.tile_pool(name="accs", bufs=1))

        # warm the ACT function-table (its ~2.7us load overlaps input DMA)
        warm = consts.tile([1, 16], F16)
        nc.vector.memset(warm[:], 0.0)
        nc.scalar.copy(warm[:], warm[:])

        lhsT_sb = consts.tile([K, HALF], BF16)
        rhs_sb = consts.tile([K, VCOLS], BF16)
        # first super-tile's operands land first so matmuls start early
        nc.sync.dma_start(lhsT_sb[:, 0:TPG * 128], lhsT[:, 0:TPG * 128])
        nc.sync.dma_start(rhs_sb[:, 0:1024], rhs[:, 0:1024])
        nc.sync.dma_start(rhs_sb[:, 1024:VCOLS], rhs[:, 1024:VCOLS])
        nc.sync.dma_start(lhsT_sb[:, TPG * 128:HALF], lhsT[:, TPG * 128:HALF])

        rowminb = accs.tile([128, ROW_TILES], F16)

        for j in range(NGRP):
            ps = psum_pool.tile([128, TPG * 512], F32)
            for t in range(TPG):
                i = TPG * j + t
                nc.tensor.matmul(
                    ps[:, t * 512:(t + 1) * 512],
                    lhsT_sb[:, i * 128:(i + 1) * 128],
                    rhs_sb[:, i * 128:i * 128 + BAND],
                    start=True, stop=True,
                )
            cst = cast_pool.tile([128, TPG, 512], F16)
            nc.scalar.copy(cst[:], ps[:])
            # per-tile row minima: one batched reduce emits TPG columns
            nc.vector.tensor_reduce(
                rowminb[:, TPG * j:TPG * (j + 1)], cst[:],
                axis=mybir.AxisListType.X, op=MIN,
            )
            nc.sync.dma_start(o_ct[j], cst[:])
        nc.sync.dma_start(o_rm, rowminb[:])
    if split_waits:
        split_multi_waits(nc)  # CoreSim can't model the injected waits
    return nc


_program_cache = {}


def _get_program():
    if "p" not in _program_cache:
        _program_cache["p"] = build_program()
    return _program_cache["p"]


def enable_profiling():
    """Wire up the NTFF profiling hook (the image's antenv lacks
    antenv.axon_hooks) and neuter the credential-requiring artifact upload.
    Needed only when tracing (BASS_TRACE=1); harmless otherwise."""
    import types
    import antenv
    import concourse.bass_utils as _bu

    if "antenv.axon_hooks" not in sys.modules:
        hooks = types.ModuleType("antenv.axon_hooks")
        hooks._h = None
        hooks.set_axon_ntff_profile_hook = lambda h: setattr(hooks, "_h", h)
        hooks.get_axon_ntff_profile_hook = lambda: hooks._h
        sys.modules["antenv.axon_hooks"] = hooks
        antenv.axon_hooks = hooks
        try:
            from trn_agent_boot.trn_boot import _ntff_profile_via_ctypes

            hooks.set_axon_ntff_profile_hook(
                _ntff_profile_via_ctypes("/opt/axon/libaxon_pjrt.so")
            )
        except Exception:
            pass
    _bu.upload_artifacts = lambda tmpdir: f"local:{tmpdir}"


if os.environ.get("BASS_TRACE"):
    try:
        enable_profiling()
    except Exception:
        pass


def _split3(x):
    x1 = x.astype(BF16NP)
    r = x - x1.astype(np.float32)
    x2 = r.astype(BF16NP)
    x3 = (r - x2.astype(np.float32)).astype(BF16NP)
    return x1, x2, x3


def _aug_dekker(t, s, s_sq):
    """K=24 bf16 3-way-split augmentation. Each fp32 value a = a1+a2+a3 in
    bf16 parts; products kept to O(2^-27): a1b1, a1b2, a2b1, a1b3, a3b1,
    a2b2. PE cost is free-dim cycles only, so K=24 runs as fast as K=5."""
    rows, cols = t.shape[0], s.shape[0]
    t1, t2, t3 = _split3(t)
    s1, s2, s3 = _split3(-2.0 * s)
    n1, n2, n3 = _split3((t * t).sum(axis=1))
    m1, m2, m3 = _split3(s_sq)
    one = np.ones((), BF16NP)
    lhsT = np.empty((24, rows), BF16NP)
    for j, part in enumerate((t1, t1, t2, t1, t3, t2)):
        lhsT[3 * j:3 * j + 3] = part.T
    lhsT[18] = n1
    lhsT[19] = n2
    lhsT[20] = n3
    lhsT[21:24] = one
    rhs = np.empty((24, cols), BF16NP)
    for j, part in enumerate((s1, s2, s1, s3, s1, s2)):
        rhs[3 * j:3 * j + 3] = part.T
    rhs[18:21] = one
    rhs[21] = m1
    rhs[22] = m2
    rhs[23] = m3
    return lhsT, rhs


def _prep(template, source):
    """Sort per batch by z; build per-core shifted+padded source windows."""
    template = np.asarray(template, dtype=np.float32)
    source = np.asarray(source, dtype=np.float32)
    tb_all, sb_all, in_maps = [], [], []
    for b in range(B):
        tb = template[b][np.argsort(template[b][:, 2], kind="stable")]
        sb = source[b][np.argsort(source[b][:, 2], kind="stable")]
        tb_all.append(tb)
        sb_all.append(sb)
    for c in range(NCORES):
        b, h = c // 2, c % 2
        tb, sb = tb_all[b], sb_all[b]
        t = tb[h * HALF:(h + 1) * HALF]
        lo = HALF * h - W
        idx = np.arange(lo, lo + VCOLS)
        valid = (idx >= 0) & (idx < M)
        s = np.where(valid[:, None], sb[np.clip(idx, 0, M - 1)], 0.0)
        s_sq = np.where(valid, (s * s).sum(axis=1), SENT_SQ).astype(np.float32)
        lhsT, rhs = _aug_dekker(t, s.astype(np.float32), s_sq)
        in_maps.append(
            {"lhsT_aug": np.ascontiguousarray(lhsT),
             "rhs_aug": np.ascontiguousarray(rhs)}
        )
    return tb_all, sb_all, in_maps


last_results = None  # BassKernelResults of the most recent kernel() call


def kernel(template, source):
    global last_results
    nc = _get_program()
    tb_all, sb_all, in_maps = _prep(template, source)
    res = run_bass_kernel_spmd(nc, in_maps, list(range(NCORES)))
    last_results = res

    per_batch = np.zeros(B, dtype=np.float64)
    for b in range(B):
        tb = tb_all[b].astype(np.float64)
        sb = sb_all[b].astype(np.float64)
        ztb, zsb = tb[:, 2], sb[:, 2]

        rowmin = np.empty(N, dtype=np.float64)
        colmin = np.full(M, np.inf)
        # virtual-column index per (tile, band position)
        idxm = 128 * np.arange(ROW_TILES)[:, None] + np.arange(BAND)[None, :]
        for h in range(2):
            r = res.results[2 * b + h]
            rm = r["out_rowmin"].astype(np.float64)  # [128, 32]
            # column j of rm = per-row minima of tile j (rows = partitions)
            rowmin[h * HALF:(h + 1) * HALF] = rm.T.reshape(-1)
            # [NGRP,128,TPG*512] -> per-tile blocks -> min over partitions
            ct = r["out_cst"].astype(np.float64)
            tiles = ct.reshape(NGRP, 128, TPG, 512).transpose(0, 2, 1, 3)
            colpart = tiles.reshape(ROW_TILES, 128, BAND).min(axis=1)
            lo = HALF * h - W
            rank = idxm + lo
            valid = (rank >= 0) & (rank < M)
            np.minimum.at(colmin, np.clip(rank, 0, M - 1).ravel(),
                          np.where(valid, colpart, np.inf).ravel())

        # --- exactness patch: rows whose banded min could be beaten by an
        # out-of-band source point (d_outside >= z_gap^2) ---
        g = np.arange(N) // 128
        band_lo = HALF * (g // ROW_TILES) - W + 128 * (g % ROW_TILES)
        band_hi = band_lo + BAND
        gap_lo = np.where(band_lo > 0,
                          np.abs(ztb - zsb[np.clip(band_lo - 1, 0, M - 1)]),
                          np.inf)
        gap_hi = np.where(band_hi < M,
                          np.abs(zsb[np.clip(band_hi, 0, M - 1)] - ztb),
                          np.inf)
        gap2 = np.minimum(gap_lo, gap_hi) ** 2
        flag_r = np.where(rowmin > gap2 * 0.95 - 1e-4)[0]
        if len(flag_r):
            d = ((tb[flag_r][:, None, :] - sb[None, :, :]) ** 2).sum(-1)
            rowmin[flag_r] = d.min(axis=1)

        # --- exactness patch: columns (symmetric) ---
        tile_lo = HALF * (np.arange(2 * ROW_TILES) // ROW_TILES) - W \
            + 128 * (np.arange(2 * ROW_TILES) % ROW_TILES)
        v = np.arange(M)
        cov = (v[None, :] >= tile_lo[:, None]) & \
              (v[None, :] < tile_lo[:, None] + BAND)  # [64, M]
        n_lo = np.argmax(cov, axis=0) * 128
        n_hi = (len(cov) - 1 - np.argmax(cov[::-1], axis=0)) * 128 + 128
        zg_lo = np.where(n_lo > 0,
                         np.abs(zsb - ztb[np.clip(n_lo - 1, 0, N - 1)]),
                         np.inf)
        zg_hi = np.where(n_hi < N,
                         np.abs(ztb[np.clip(n_hi, 0, N - 1)] - zsb),
                         np.inf)
        gap2c = np.minimum(zg_lo, zg_hi) ** 2
        flag_c = np.where(colmin > gap2c * 0.95 - 1e-4)[0]
        if len(flag_c):
            d = ((tb[None, :, :] - sb[flag_c][:, None, :]) ** 2).sum(-1)
            colmin[flag_c] = d.min(axis=1)

        per_batch[b] = rowmin.mean() + colmin.mean()
    return np.float32(per_batch.mean())


# revision 16
# speedup vs baseline: 9.1017x; 1.0380x over previous
"""Chamfer distance loss kernel for Trainium2 (8 NeuronCores).

Problem: template/source (4, 8192, 3) f32. For each batch b:
  d[n,m] = |t_n|^2 - 2 t_n.s_m + |s_m|^2
  loss_b = mean_n min_m d + mean_m min_n d ; output = mean_b loss_b (scalar).

Strategy: z-sorted banding. Both point sets are sorted by z on the host
(per batch). A 128-row template tile then only needs a contiguous band
of ~1024 z-sorted source columns: nearest neighbors are z-close for all
but a handful of radial outliers. This cuts the distance-matrix volume
8x (32 tiles x 1024 cols per core instead of 32 x 8192).

The banding is made EXACT by a host-side patch: a banded row-min can
only be wrong if the true nearest neighbor lies outside the band, which
implies d >= (z-gap to the band edge)^2. Rows/cols whose banded min
exceeds that bound are recomputed exactly in numpy (a few hundred per
batch, milliseconds). Means are permutation-invariant so the sort needs
no undoing.

Sharding: core c = (batch c//2, sorted-template-half c%2); each core's
source input is a pre-shifted 5120-rank window of the sorted source
(rank range [4096h-448, 4096h+4672)) padded with far-away sentinel
points, which keeps one SPMD program valid for both halves: tile i
always scans virtual columns [128i, 128i+1024).

Per core, each [128 x 512] distance tile is produced in PSUM by ONE
augmented matmul: d = |t|^2 - 2 t.s + |s|^2 as a K=24 contraction of
bf16 3-way value splits (fp32-grade distances at 1 cycle/row). ScalarE
casts each 1024-col PSUM group to fp16 once; VectorE does the row-min
reduce and the column-min accumulate in fp16 2x packed mode. Column
minima ship to HBM in 1024-col chunks as soon as their last touching
tile has passed, overlapping output DMA with compute.
"""
import os
import sys

sys.path.insert(0, "/opt/trn_rl_repo")

from contextlib import ExitStack

import numpy as np

import concourse.bass as bass
import concourse.tile as tile
from concourse import mybir
from concourse.bass_utils import run_bass_kernel_spmd

# ---------------------------------------------------------------------------
# The walrus build in this container rejects instructions carrying more than
# one sync-wait command. After Tile scheduling, split any multi-wait
# instruction: keep the first wait on it and hoist the rest onto standalone
# EventSemaphore instructions inserted just before it (same engine, so
# per-engine program order makes the waits execute first).
import bass_rust as _br


def split_multi_waits(nc):
    n_new = 0
    for fn in nc.m.functions:
        for blk in fn.blocks:
            insts = list(blk.instructions)
            out = []
            changed = False
            for inst in insts:
                si = inst.sync_info
                waits = list(si.on_wait) if si is not None and si.on_wait else []
                if len(waits) > 1:
                    for w in waits[:-1]:
                        ev = _br.InstEventSemaphore(
                            name=f"I-waitsplit-{n_new}", ins=[], outs=[]
                        )
                        n_new += 1
                        ev.engine = inst.engine
                        ev.sync_info = _br.SyncInfo(on_wait=[w], on_update=[])
                        out.append(ev)
                    si.on_wait = [waits[-1]]
                    changed = True
                out.append(inst)
            if changed:
                blk.instructions = out
# ---------------------------------------------------------------------------

import ml_dtypes

F32 = mybir.dt.float32
F16 = mybir.dt.float16
BF16 = mybir.dt.bfloat16
MIN = mybir.AluOpType.min
BF16NP = ml_dtypes.bfloat16

B, N, M, D = 4, 8192, 8192, 3
HALF = N // 2       # template rows per core
NCORES = 8
K = 24              # dekker-split contraction depth
W = 192             # band half-width in source ranks
BAND = 512          # columns per tile band (= 128 + 2*W)
VCOLS = HALF + 2 * W + 128  # virtual source columns per core = 6144
ROW_TILES = HALF // 128     # 32
BIG = 60000.0       # fp16-safe "+inf" for min accumulators
SENT_SQ = 30000.0   # sentinel |s|^2: d at padded columns never wins a min


# row tiles per PSUM group: small lead-in groups fill the pipeline sooner,
# a small tail group shrinks the final exposed DMA
GROUPS = [1, 1, 2] + [4] * 6 + [2, 2]
assert sum(GROUPS) == ROW_TILES


def build_program(split_waits=True):
    nc = bass.Bass("TRN2", target_bir_lowering=False, debug=False)
    lhsT = nc.dram_tensor("lhsT_aug", [K, HALF], BF16, kind="ExternalInput").ap()
    rhs = nc.dram_tensor("rhs_aug", [K, VCOLS], BF16, kind="ExternalInput").ap()
    o_rm = nc.dram_tensor(
        "out_rowmin", [128, ROW_TILES], F16, kind="ExternalOutput"
    ).ap()
    # the raw banded fp16 distance blocks; the host derives column minima
    # from them (each column is covered by only ~4 tiles)
    o_ct = nc.dram_tensor(
        "out_cst", [128, ROW_TILES * 512], F16, kind="ExternalOutput"
    ).ap()

    with tile.TileContext(nc) as tc, ExitStack() as ctx:
        consts = ctx.enter_context(tc.tile_pool(name="consts", bufs=1))
        psum_pool = ctx.enter_context(tc.tile_pool(name="psum", bufs=2, space="PSUM"))
        cast_pool = ctx.enter_context(tc.tile_pool(name="cast", bufs=3))
        rfold_pool = ctx.enter_context(tc.tile_pool(name="rfold", bufs=2))
        accs = ctx.enter_context(tc.tile_pool(name="accs", bufs=1))

        # warm the ACT function-table (its ~2.7us load overlaps input DMA)
        warm = consts.tile([1, 16], F16)
        nc.vector.memset(warm[:], 0.0)
        nc.scalar.copy(warm[:], warm[:])

        lhsT_sb = consts.tile([K, HALF], BF16)
        rhs_sb = consts.tile([K, VCOLS], BF16)
        # first groups' operands land first so matmuls start early
        nc.sync.dma_start(lhsT_sb[:, 0:512], lhsT[:, 0:512])
        nc.sync.dma_start(rhs_sb[:, 0:1024], rhs[:, 0:1024])
        nc.sync.dma_start(rhs_sb[:, 1024:VCOLS], rhs[:, 1024:VCOLS])
        nc.sync.dma_start(lhsT_sb[:, 512:HALF], lhsT[:, 512:HALF])

        rowminb = accs.tile([128, ROW_TILES], F16)

        i0 = 0
        for tpg in GROUPS:
            ps = psum_pool.tile([128, tpg * 512], F32)
            for t in range(tpg):
                i = i0 + t
                nc.tensor.matmul(
                    ps[:, t * 512:(t + 1) * 512],
                    lhsT_sb[:, i * 128:(i + 1) * 128],
                    rhs_sb[:, i * 128:i * 128 + BAND],
                    start=True, stop=True,
                )
            cst = cast_pool.tile([128, tpg, 512], F16)
            nc.scalar.copy(cst[:], ps[:])
            # per-tile row minima: fold 512->128 with 2x-mode tensor_tensor
            # ops, then one batched 1x reduce emits tpg columns at once
            rf = rfold_pool.tile([128, tpg, 256], F16)
            nc.vector.tensor_tensor(
                rf[:], cst[:, :, 0:256], cst[:, :, 256:512], op=MIN)
            nc.vector.tensor_tensor(
                rf[:, :, 0:128], rf[:, :, 0:128], rf[:, :, 128:256], op=MIN)
            nc.vector.tensor_reduce(
                rowminb[:, i0:i0 + tpg], rf[:, :, 0:128],
                axis=mybir.AxisListType.X, op=MIN,
            )
            nc.sync.dma_start(
                o_ct[:, i0 * 512:(i0 + tpg) * 512], cst[:])
            i0 += tpg
        nc.sync.dma_start(o_rm, rowminb[:])
    if split_waits:
        split_multi_waits(nc)  # CoreSim can't model the injected waits
    return nc


_program_cache = {}


def _get_program():
    if "p" not in _program_cache:
        _program_cache["p"] = build_program()
    return _program_cache["p"]


def enable_profiling():
    """Wire up the NTFF profiling hook (the image's antenv lacks
    antenv.axon_hooks) and neuter the credential-requiring artifact upload.
    Needed only when tracing (BASS_TRACE=1); harmless otherwise."""
    import types
    import antenv
    import concourse.bass_utils as _bu

    if "antenv.axon_hooks" not in sys.modules:
        hooks = types.ModuleType("antenv.axon_hooks")
        hooks._h = None
        hooks.set_axon_ntff_profile_hook = lambda h: setattr(hooks, "_h", h)
        hooks.get_axon_ntff_profile_hook = lambda: hooks._h
        sys.modules["antenv.axon_hooks"] = hooks
        antenv.axon_hooks = hooks
        try:
            from trn_agent_boot.trn_boot import _ntff_profile_via_ctypes

            hooks.set_axon_ntff_profile_hook(
                _ntff_profile_via_ctypes("/opt/axon/libaxon_pjrt.so")
            )
        except Exception:
            pass
    _bu.upload_artifacts = lambda tmpdir: f"local:{tmpdir}"


if os.environ.get("BASS_TRACE"):
    try:
        enable_profiling()
    except Exception:
        pass


def _split3(x):
    x1 = x.astype(BF16NP)
    r = x - x1.astype(np.float32)
    x2 = r.astype(BF16NP)
    x3 = (r - x2.astype(np.float32)).astype(BF16NP)
    return x1, x2, x3


def _aug_dekker(t, s, s_sq):
    """K=24 bf16 3-way-split augmentation. Each fp32 value a = a1+a2+a3 in
    bf16 parts; products kept to O(2^-27): a1b1, a1b2, a2b1, a1b3, a3b1,
    a2b2. PE cost is free-dim cycles only, so K=24 runs as fast as K=5."""
    rows, cols = t.shape[0], s.shape[0]
    t1, t2, t3 = _split3(t)
    s1, s2, s3 = _split3(-2.0 * s)
    n1, n2, n3 = _split3((t * t).sum(axis=1))
    m1, m2, m3 = _split3(s_sq)
    one = np.ones((), BF16NP)
    lhsT = np.empty((24, rows), BF16NP)
    for j, part in enumerate((t1, t1, t2, t1, t3, t2)):
        lhsT[3 * j:3 * j + 3] = part.T
    lhsT[18] = n1
    lhsT[19] = n2
    lhsT[20] = n3
    lhsT[21:24] = one
    rhs = np.empty((24, cols), BF16NP)
    for j, part in enumerate((s1, s2, s1, s3, s1, s2)):
        rhs[3 * j:3 * j + 3] = part.T
    rhs[18:21] = one
    rhs[21] = m1
    rhs[22] = m2
    rhs[23] = m3
    return lhsT, rhs


def _prep(template, source):
    """Sort per batch by z; build per-core shifted+padded source windows."""
    template = np.asarray(template, dtype=np.float32)
    source = np.asarray(source, dtype=np.float32)
    tb_all, sb_all, in_maps = [], [], []
    for b in range(B):
        tb = template[b][np.argsort(template[b][:, 2], kind="stable")]
        sb = source[b][np.argsort(source[b][:, 2], kind="stable")]
        tb_all.append(tb)
        sb_all.append(sb)
    for c in range(NCORES):
        b, h = c // 2, c % 2
        tb, sb = tb_all[b], sb_all[b]
        t = tb[h * HALF:(h + 1) * HALF]
        lo = HALF * h - W
        idx = np.arange(lo, lo + VCOLS)
        valid = (idx >= 0) & (idx < M)
        s = np.where(valid[:, None], sb[np.clip(idx, 0, M - 1)], 0.0)
        s_sq = np.where(valid, (s * s).sum(axis=1), SENT_SQ).astype(np.float32)
        lhsT, rhs = _aug_dekker(t, s.astype(np.float32), s_sq)
        in_maps.append(
            {"lhsT_aug": np.ascontiguousarray(lhsT),
             "rhs_aug": np.ascontiguousarray(rhs)}
        )
    return tb_all, sb_all, in_maps


last_results = None  # BassKernelResults of the most recent kernel() call


def kernel(template, source):
    global last_results
    nc = _get_program()
    tb_all, sb_all, in_maps = _prep(template, source)
    res = run_bass_kernel_spmd(nc, in_maps, list(range(NCORES)))
    last_results = res

    per_batch = np.zeros(B, dtype=np.float64)
    for b in range(B):
        tb = tb_all[b].astype(np.float64)
        sb = sb_all[b].astype(np.float64)
        ztb, zsb = tb[:, 2], sb[:, 2]

        rowmin = np.empty(N, dtype=np.float64)
        colmin = np.full(M, np.inf)
        # virtual-column index per (tile, band position)
        idxm = 128 * np.arange(ROW_TILES)[:, None] + np.arange(BAND)[None, :]
        for h in range(2):
            r = res.results[2 * b + h]
            rm = r["out_rowmin"].astype(np.float64)  # [128, 32]
            # column j of rm = per-row minima of tile j (rows = partitions)
            rowmin[h * HALF:(h + 1) * HALF] = rm.T.reshape(-1)
            # [128, 32*512] -> per-tile blocks -> min over partitions
            ct = r["out_cst"].astype(np.float64)
            tiles = ct.reshape(128, ROW_TILES, 512).transpose(1, 0, 2)
            colpart = tiles.min(axis=1)
            lo = HALF * h - W
            rank = idxm + lo
            valid = (rank >= 0) & (rank < M)
            np.minimum.at(colmin, np.clip(rank, 0, M - 1).ravel(),
                          np.where(valid, colpart, np.inf).ravel())

        # --- exactness patch: rows whose banded min could be beaten by an
        # out-of-band source point (d_outside >= z_gap^2) ---
        g = np.arange(N) // 128
        band_lo = HALF * (g // ROW_TILES) - W + 128 * (g % ROW_TILES)
        band_hi = band_lo + BAND
        gap_lo = np.where(band_lo > 0,
                          np.abs(ztb - zsb[np.clip(band_lo - 1, 0, M - 1)]),
                          np.inf)
        gap_hi = np.where(band_hi < M,
                          np.abs(zsb[np.clip(band_hi, 0, M - 1)] - ztb),
                          np.inf)
        gap2 = np.minimum(gap_lo, gap_hi) ** 2
        flag_r = np.where(rowmin > gap2 * 0.95 - 1e-4)[0]
        if len(flag_r):
            d = ((tb[flag_r][:, None, :] - sb[None, :, :]) ** 2).sum(-1)
            rowmin[flag_r] = d.min(axis=1)

        # --- exactness patch: columns (symmetric) ---
        tile_lo = HALF * (np.arange(2 * ROW_TILES) // ROW_TILES) - W \
            + 128 * (np.arange(2 * ROW_TILES) % ROW_TILES)
        v = np.arange(M)
        cov = (v[None, :] >= tile_lo[:, None]) & \
              (v[None, :] < tile_lo[:, None] + BAND)  # [64, M]
        n_lo = np.argmax(cov, axis=0) * 128
        n_hi = (len(cov) - 1 - np.argmax(cov[::-1], axis=0)) * 128 + 128
        zg_lo = np.where(n_lo > 0,
                         np.abs(zsb - ztb[np.clip(n_lo - 1, 0, N - 1)]),
                         np.inf)
        zg_hi = np.where(n_hi < N,
                         np.abs(ztb[np.clip(n_hi, 0, N - 1)] - zsb),
                         np.inf)
        gap2c = np.minimum(zg_lo, zg_hi) ** 2
        flag_c = np.where(colmin > gap2c * 0.95 - 1e-4)[0]
        if len(flag_c):
            d = ((tb[None, :, :] - sb[flag_c][:, None, :]) ** 2).sum(-1)
            colmin[flag_c] = d.min(axis=1)

        per_batch[b] = rowmin.mean() + colmin.mean()
    return np.float32(per_batch.mean())
